# revision 27
# baseline (speedup 1.0000x reference)
"""Single-head attention layer (Q/K/V proj + softmax(QK^T)V) on 8 trn2 NeuronCores.

Strategy: pure data-parallel over batch B=16 -> 2 batches per core, zero
communication. All matmuls run in float32r (fp32 storage, rounded fp32 PE mode,
1 cycle/row at free-dim>=512 => bf16-rate with ~tf32 precision).

Per core, per batch (x_b: [2048, 512]):
  1. x^T via PE transposes (d on partitions), rounded to f32r.
  2. q^T, k^T = (Wq/Wk)^T-contract projections in channel-major layout
     [e, token]; bias added per-partition during PSUM->SBUF copy.
     v = x @ Wv + bv in token-major layout [token, e].
  3. Scores computed transposed: S^T[j, i] = sum_e k^T[e,j] q^T[e,i],
     per i-block of 512 queries; exp (no max subtraction: |S| <~ 50, safe
     in fp32) written straight to SBUF as f32r => P^T ready for PV matmul.
  4. Softmax denominators via ones-vector matmul (sums over j partitions),
     tiny PE transposes to land them on i-partitions, DVE reciprocal.
  5. out[i_tile] = P^T.T @ v accumulated over 16 j-tiles; normalization
     folded into the PSUM->SBUF copy (per-partition scale), DMA to DRAM.
"""

import os

import numpy as np

try:  # NTFF profiling hook is optional; without it, disable tracing so a
    # stray BASS_TRACE=1 in the environment cannot crash the run.
    from antenv.axon_hooks import get_axon_ntff_profile_hook  # noqa: F401
except ImportError:
    os.environ.setdefault("BASS_NEVER_TRACE", "1")

import concourse.bass as bass
import concourse.tile as tile
from concourse import bacc, mybir
from concourse.bass_utils import run_bass_kernel_spmd
from concourse.masks import make_identity

f32 = mybir.dt.float32
f32r = mybir.dt.float32r
bf16 = mybir.dt.bfloat16

B, N, D = 16, 2048, 512
NCORES = 8
PB = B // NCORES  # batches per core
NT = N // 128  # 16 token tiles
DC = D // 128  # 4 channel chunks of 128
NIB = N // 512  # 4 query blocks of 512
JT = NT  # 16 key tiles


def build():
    nc = bacc.Bacc("TRN2", target_bir_lowering=False, debug=False)

    x = nc.dram_tensor("x", [PB, N, D], f32, kind="ExternalInput")
    Wq = nc.dram_tensor("Wq", [D, D], f32, kind="ExternalInput")
    bq = nc.dram_tensor("bq", [D], f32, kind="ExternalInput")
    Wk = nc.dram_tensor("Wk", [D, D], f32, kind="ExternalInput")
    bk = nc.dram_tensor("bk", [D], f32, kind="ExternalInput")
    Wv = nc.dram_tensor("Wv", [D, D], f32, kind="ExternalInput")
    bv = nc.dram_tensor("bv", [D], f32, kind="ExternalInput")
    out = nc.dram_tensor("out", [PB, N, D], f32, kind="ExternalOutput")

    with tile.TileContext(nc) as tc:
        with (
            tc.tile_pool(name="singles", bufs=1) as singles,
            tc.tile_pool(name="psbank", bufs=4, space="PSUM") as psbank,
            tc.tile_pool(name="pstrans", bufs=1, space="PSUM") as pstrans,
            tc.tile_pool(name="pssums", bufs=1, space="PSUM") as pssums,
            tc.tile_pool(name="pspv", bufs=2, space="PSUM") as pspv,
            tc.tile_pool(name="spool", bufs=1) as spool,
            tc.tile_pool(name="xstage", bufs=5) as xstage,
            tc.tile_pool(name="rpool", bufs=1) as rpool,
        ):
            ident = singles.tile([128, 128], f32)
            make_identity(nc, ident[:])
            ones_f32 = singles.tile([128, 1], f32)
            nc.vector.memset(ones_f32[:], 1.0)
            ones = singles.tile([128, 1], f32r)
            nc.vector.tensor_copy(ones[:], ones_f32[:])

            # --- weights/biases load; emitted AFTER batch-0 x loads so the
            #     PE can start transposing x while weights stream in.
            wb = {}

            def load_weights():
                # stage weight chunks through the shared xstage slots
                for W in (Wv, Wq, Wk):
                    wr = singles.tile([128, DC, D], f32r, tag=f"w_{W.name}")
                    for dc in range(DC):
                        stage = xstage.tile([128, D], f32, tag="xs")
                        nc.sync.dma_start(
                            out=stage[:],
                            in_=W[dc * 128 : (dc + 1) * 128, :],
                        )
                        nc.vector.tensor_copy(wr[:, dc, :], stage[:])
                    wb[W.name] = wr
                # biases: bv (needed first) broadcast to all partitions;
                # bq/bk as [128, dc] (channel on partitions)
                bv_bc = singles.tile([128, D], f32)
                bv_ap = bv[:]
                bv_bcast = bass.AP(
                    tensor=bv_ap.tensor, offset=bv_ap.offset, ap=[[0, 128], *bv_ap.ap]
                )
                nc.sync.dma_start(out=bv_bc[:], in_=bv_bcast)
                bqT = singles.tile([128, DC], f32)
                nc.sync.dma_start(
                    out=bqT[:], in_=bq[:].rearrange("(dc p) -> p dc", p=128)
                )
                bkT = singles.tile([128, DC], f32)
                nc.sync.dma_start(
                    out=bkT[:], in_=bk[:].rearrange("(dc p) -> p dc", p=128)
                )
                wb["bqT"], wb["bkT"], wb["bv_bc"] = bqT, bkT, bv_bc

            for b in range(PB):
                with (
                    tc.tile_pool(name=f"qkv{b}", bufs=1) as qkv_pool,
                    tc.tile_pool(name=f"pT{b}", bufs=1) as pt_pool,
                    tc.tile_pool(name=f"red{b}", bufs=1) as red_pool,
                    tc.tile_pool(name=f"ostage{b}", bufs=2) as ostage,
                    tc.tile_pool(name=f"xT{b}", bufs=1) as xt_pool,
                ):
                    qT = qkv_pool.tile([128, DC, N], f32r, tag="qT")
                    kT = qkv_pool.tile([128, DC, N], f32r, tag="kT")
                    vv = qkv_pool.tile([128, NT, D], f32r, tag="v")

                    # --- phase A: x load, transpose, projections
                    if True:
                        xT = xt_pool.tile([128, DC, N], f32r)
                        # interleave per window of 4 token tiles (= one
                        # 512-wide projection block): DMA + transpose the
                        # window, then run its projections while the next
                        # window streams in. DMAs split per dc chunk so 4
                        # queues work each tile (lower latency per tile).
                        def stage_window(w):
                            for it in range(w * 4, w * 4 + 4):
                                xs = xstage.tile([128, D], f32, tag="xs")
                                tsl = slice(it * 128, (it + 1) * 128)
                                for dc in range(DC):
                                    csl = slice(dc * 128, (dc + 1) * 128)
                                    nc.sync.dma_start(
                                        out=xs[:, csl], in_=x[b, tsl, csl]
                                    )
                                ps = psbank.tile([128, DC, 128], f32, tag="bank")
                                for dc in range(DC):
                                    nc.tensor.transpose(
                                        ps[:, dc, :],
                                        xs[:, dc * 128 : (dc + 1) * 128],
                                        ident[:],
                                    )
                                nc.scalar.copy(xT[:, :, tsl], ps[:])

                        # pass 1: per window, transpose + v-projections (only
                        # Wv needed -> Wq/Wk keep streaming in meanwhile)
                        for ib in range(NIB):
                            stage_window(ib)
                            if b == 0 and ib == 0:
                                load_weights()
                            wq_r, wk_r, wv_r = wb["Wq"], wb["Wk"], wb["Wv"]
                            bqT, bkT, bv_bc = wb["bqT"], wb["bkT"], wb["bv_bc"]
                            for jt in range(ib * 4, ib * 4 + 4):
                                jsl = slice(jt * 128, (jt + 1) * 128)
                                pv = psbank.tile([128, 512], f32, tag="bank")
                                for dc in range(DC):
                                    nc.tensor.matmul(
                                        pv[:],
                                        xT[:, dc, jsl],
                                        wv_r[:, dc, :],
                                        start=(dc == 0),
                                        stop=(dc == DC - 1),
                                    )
                                nc.vector.tensor_add(vv[:, jt, :], pv[:], bv_bc[:])
                        # pass 2: q^T/k^T projections
                        for ib in range(NIB):
                            isl = slice(ib * 512, (ib + 1) * 512)
                            # q^T, k^T: [e(128 part), i] = sum_d W[d,e] x^T[d,i]
                            for ec in range(DC):
                                esl = slice(ec * 128, (ec + 1) * 128)
                                pq = psbank.tile([128, 512], f32, tag="bank")
                                for dc in range(DC):
                                    nc.tensor.matmul(
                                        pq[:],
                                        wq_r[:, dc, esl],
                                        xT[:, dc, isl],
                                        start=(dc == 0),
                                        stop=(dc == DC - 1),
                                    )
                                nc.vector.tensor_scalar_add(
                                    qT[:, ec, isl], pq[:], bqT[:, ec : ec + 1]
                                )
                                pk = psbank.tile([128, 512], f32, tag="bank")
                                for dc in range(DC):
                                    nc.tensor.matmul(
                                        pk[:],
                                        wk_r[:, dc, esl],
                                        xT[:, dc, isl],
                                        start=(dc == 0),
                                        stop=(dc == DC - 1),
                                    )
                                nc.vector.tensor_scalar_add(
                                    kT[:, ec, isl], pk[:], bkT[:, ec : ec + 1]
                                )

                    # --- phase B: attention, one block of 512 queries at a time
                    if True:
                        for ib in range(NIB):
                            isl = slice(ib * 512, (ib + 1) * 512)
                            pT = pt_pool.tile([128, JT, 512], f32r)
                            for jt in range(JT):
                                jsl = slice(jt * 128, (jt + 1) * 128)
                                ps = psbank.tile([128, 512], f32, tag="bank")
                                for ec in range(DC):
                                    nc.tensor.matmul(
                                        ps[:],
                                        kT[:, ec, jsl],
                                        qT[:, ec, isl],
                                        start=(ec == 0),
                                        stop=(ec == DC - 1),
                                    )
                                nc.scalar.activation(
                                    pT[:, jt, :],
                                    ps[:],
                                    mybir.ActivationFunctionType.Exp,
                                )
                            # softmax denominators: s[1, i] = sum_j P^T[j, i].
                            # Pre-reduce 16 -> 4 tiles on DVE (idle during
                            # attention) to cut the PE ones-matmul count 4x.
                            red = red_pool.tile([128, 2, 512], f32r)
                            for g in range(2):
                                nc.vector.tensor_add(
                                    red[:, g, :],
                                    pT[:, 8 * g, :],
                                    pT[:, 8 * g + 1, :],
                                )
                                for j in range(8 * g + 2, 8 * g + 8):
                                    nc.vector.tensor_add(
                                        red[:, g, :], red[:, g, :], pT[:, j, :]
                                    )
                            sums_p = pssums.tile([1, 512], f32)
                            for g in range(2):
                                nc.tensor.matmul(
                                    sums_p[:],
                                    ones[:],
                                    red[:, g, :],
                                    start=(g == 0),
                                    stop=(g == 1),
                                )
                            s_sb = spool.tile([1, 512], f32)
                            nc.vector.tensor_copy(s_sb[:], sums_p[:])
                            st_p = pstrans.tile([128, 4], f32)
                            for c in range(4):
                                nc.tensor.transpose(
                                    st_p[:, c : c + 1],
                                    s_sb[0:1, c * 128 : (c + 1) * 128],
                                    ident[0:1, 0:1],
                                )
                            r_sb = rpool.tile([128, 4], f32, tag="r")
                            nc.vector.reciprocal(r_sb[:], st_p[:])

                            # out[i_sub] = (P^T)^T @ v, scaled by 1/s
                            for isub in range(4):
                                po = pspv.tile([128, 512], f32)
                                for jt in range(JT):
                                    nc.tensor.matmul(
                                        po[:],
                                        pT[:, jt, isub * 128 : (isub + 1) * 128],
                                        vv[:, jt, :],
                                        start=(jt == 0),
                                        stop=(jt == JT - 1),
                                    )
                                ob = ostage.tile([128, 512], f32, tag="ob")
                                nc.scalar.mul(ob[:], po[:], r_sb[:, isub : isub + 1])
                                t0 = ib * 512 + isub * 128
                                nc.sync.dma_start(
                                    out=out[b, t0 : t0 + 128, :], in_=ob[:]
                                )
    nc.finalize()
    return nc


_built = None


def kernel(x, Wq, bq, Wk, bk, Wv, bv):
    global _built
    x = np.ascontiguousarray(np.asarray(x, dtype=np.float32))
    ws = {
        "Wq": np.ascontiguousarray(np.asarray(Wq, dtype=np.float32)),
        "bq": np.ascontiguousarray(np.asarray(bq, dtype=np.float32)),
        "Wk": np.ascontiguousarray(np.asarray(Wk, dtype=np.float32)),
        "bk": np.ascontiguousarray(np.asarray(bk, dtype=np.float32)),
        "Wv": np.ascontiguousarray(np.asarray(Wv, dtype=np.float32)),
        "bv": np.ascontiguousarray(np.asarray(bv, dtype=np.float32)),
    }
    if _built is None:
        _built = build()
    in_maps = [
        {"x": np.ascontiguousarray(x[c * PB : (c + 1) * PB]), **ws}
        for c in range(NCORES)
    ]
    res = run_bass_kernel_spmd(_built, in_maps, core_ids=list(range(NCORES)))
    kernel.last_exec_time_ns = res.exec_time_ns
    return np.concatenate([r["out"] for r in res.results], axis=0)


kernel.last_exec_time_ns = None


# revision 29
# speedup vs baseline: 1.0754x; 1.0754x over previous
"""Single-head attention layer (Q/K/V proj + softmax(QK^T)V) on 8 trn2 NeuronCores.

Strategy: pure data-parallel over batch B=16 -> 2 batches per core, zero
communication. All matmuls run in float32r (fp32 storage, rounded fp32 PE mode,
1 cycle/row at free-dim>=512 => bf16-rate with ~tf32 precision).

Per core, per batch (x_b: [2048, 512]):
  1. x^T via PE transposes (d on partitions), rounded to f32r.
  2. q^T, k^T = (Wq/Wk)^T-contract projections in channel-major layout
     [e, token]; bias added per-partition during PSUM->SBUF copy.
     v = x @ Wv + bv in token-major layout [token, e].
  3. Scores computed transposed: S^T[j, i] = sum_e k^T[e,j] q^T[e,i],
     per i-block of 512 queries; exp (no max subtraction: |S| <~ 50, safe
     in fp32) written straight to SBUF as f32r => P^T ready for PV matmul.
  4. Softmax denominators via ones-vector matmul (sums over j partitions),
     tiny PE transposes to land them on i-partitions, DVE reciprocal.
  5. out[i_tile] = P^T.T @ v accumulated over 16 j-tiles; normalization
     folded into the PSUM->SBUF copy (per-partition scale), DMA to DRAM.
"""

import os

import numpy as np

try:  # NTFF profiling hook is optional; without it, disable tracing so a
    # stray BASS_TRACE=1 in the environment cannot crash the run.
    from antenv.axon_hooks import get_axon_ntff_profile_hook  # noqa: F401
except ImportError:
    os.environ.setdefault("BASS_NEVER_TRACE", "1")

import concourse.bass as bass
import concourse.tile as tile
from concourse import bacc, mybir
from concourse.bass_utils import run_bass_kernel_spmd
from concourse.masks import make_identity

f32 = mybir.dt.float32
f32r = mybir.dt.float32r
bf16 = mybir.dt.bfloat16

B, N, D = 16, 2048, 512
NCORES = 8
PB = B // NCORES  # batches per core
NT = N // 128  # 16 token tiles
DC = D // 128  # 4 channel chunks of 128
NIB = N // 512  # 4 query blocks of 512
JT = NT  # 16 key tiles


def build():
    nc = bacc.Bacc("TRN2", target_bir_lowering=False, debug=False)

    x = nc.dram_tensor("x", [PB, N, D], f32, kind="ExternalInput")
    Wq = nc.dram_tensor("Wq", [D, D], f32, kind="ExternalInput")
    bq = nc.dram_tensor("bq", [D], f32, kind="ExternalInput")
    Wk = nc.dram_tensor("Wk", [D, D], f32, kind="ExternalInput")
    bk = nc.dram_tensor("bk", [D], f32, kind="ExternalInput")
    Wv = nc.dram_tensor("Wv", [D, D], f32, kind="ExternalInput")
    bv = nc.dram_tensor("bv", [D], f32, kind="ExternalInput")
    out = nc.dram_tensor("out", [PB, N, D], f32, kind="ExternalOutput")

    with tile.TileContext(nc) as tc:
        with (
            tc.tile_pool(name="singles", bufs=1) as singles,
            tc.tile_pool(name="psbank", bufs=5, space="PSUM") as psbank,
            tc.tile_pool(name="pstrans", bufs=1, space="PSUM") as pstrans,
            tc.tile_pool(name="pssums", bufs=1, space="PSUM") as pssums,
            tc.tile_pool(name="pspv", bufs=1, space="PSUM") as pspv,
            tc.tile_pool(name="spool", bufs=1) as spool,
            tc.tile_pool(name="xstage", bufs=5) as xstage,
            tc.tile_pool(name="rpool", bufs=1) as rpool,
        ):
            ident = singles.tile([128, 128], f32)
            make_identity(nc, ident[:])
            ones_f32 = singles.tile([128, 1], f32)
            nc.vector.memset(ones_f32[:], 1.0)
            ones = singles.tile([128, 1], f32r)
            nc.vector.tensor_copy(ones[:], ones_f32[:])

            # --- weights/biases load; emitted AFTER batch-0 x loads so the
            #     PE can start transposing x while weights stream in.
            wb = {}

            def load_weights():
                # stage weight chunks through the shared xstage slots
                for W in (Wv, Wq, Wk):
                    wr = singles.tile([128, DC, D], f32r, tag=f"w_{W.name}")
                    for dc in range(DC):
                        stage = xstage.tile([128, D], f32, tag="xs")
                        nc.sync.dma_start(
                            out=stage[:],
                            in_=W[dc * 128 : (dc + 1) * 128, :],
                        )
                        nc.vector.tensor_copy(wr[:, dc, :], stage[:])
                    wb[W.name] = wr
                # biases: bv (needed first) broadcast to all partitions;
                # bq/bk as [128, dc] (channel on partitions)
                bv_bc = singles.tile([128, D], f32)
                bv_ap = bv[:]
                bv_bcast = bass.AP(
                    tensor=bv_ap.tensor, offset=bv_ap.offset, ap=[[0, 128], *bv_ap.ap]
                )
                nc.sync.dma_start(out=bv_bc[:], in_=bv_bcast)
                bqT = singles.tile([128, DC], f32)
                nc.sync.dma_start(
                    out=bqT[:], in_=bq[:].rearrange("(dc p) -> p dc", p=128)
                )
                bkT = singles.tile([128, DC], f32)
                nc.sync.dma_start(
                    out=bkT[:], in_=bk[:].rearrange("(dc p) -> p dc", p=128)
                )
                wb["bqT"], wb["bkT"], wb["bv_bc"] = bqT, bkT, bv_bc

            for b in range(PB):
                with (
                    tc.tile_pool(name=f"qkv{b}", bufs=1) as qkv_pool,
                    tc.tile_pool(name=f"pT{b}", bufs=1) as pt_pool,
                    tc.tile_pool(name=f"red{b}", bufs=1) as red_pool,
                    tc.tile_pool(name=f"ostage{b}", bufs=2) as ostage,
                    tc.tile_pool(name=f"xT{b}", bufs=1) as xt_pool,
                ):
                    qT = qkv_pool.tile([128, DC, N], f32r, tag="qT")
                    kT = qkv_pool.tile([128, DC, N], f32r, tag="kT")
                    vv = qkv_pool.tile([128, NT, D], f32r, tag="v")

                    # --- phase A: x load, transpose, projections
                    if True:
                        xT = xt_pool.tile([128, DC, N], f32r)
                        # interleave per window of 4 token tiles (= one
                        # 512-wide projection block): DMA + transpose the
                        # window, then run its projections while the next
                        # window streams in. DMAs split per dc chunk so 4
                        # queues work each tile (lower latency per tile).
                        def stage_window(w):
                            for it in range(w * 4, w * 4 + 4):
                                xs = xstage.tile([128, D], f32, tag="xs")
                                tsl = slice(it * 128, (it + 1) * 128)
                                for dc in range(DC):
                                    csl = slice(dc * 128, (dc + 1) * 128)
                                    nc.sync.dma_start(
                                        out=xs[:, csl], in_=x[b, tsl, csl]
                                    )
                                ps = psbank.tile([128, DC, 128], f32, tag="bank")
                                for dc in range(DC):
                                    nc.tensor.transpose(
                                        ps[:, dc, :],
                                        xs[:, dc * 128 : (dc + 1) * 128],
                                        ident[:],
                                    )
                                nc.scalar.copy(xT[:, :, tsl], ps[:])

                        for ib in range(NIB):
                            stage_window(ib)
                            if b == 0 and ib == 0:
                                load_weights()
                            wq_r, wk_r, wv_r = wb["Wq"], wb["Wk"], wb["Wv"]
                            bqT, bkT, bv_bc = wb["bqT"], wb["bkT"], wb["bv_bc"]

                            isl = slice(ib * 512, (ib + 1) * 512)
                            # v first: v(jt) needs only tile jt, so it can run
                            # while the window's later xT copies land; q/k (which
                            # need the full window) go last, stall-free.
                            for jt in range(ib * 4, ib * 4 + 4):
                                jsl = slice(jt * 128, (jt + 1) * 128)
                                pv = psbank.tile([128, 512], f32, tag="bank")
                                for dc in range(DC):
                                    nc.tensor.matmul(
                                        pv[:],
                                        xT[:, dc, jsl],
                                        wv_r[:, dc, :],
                                        start=(dc == 0),
                                        stop=(dc == DC - 1),
                                    )
                                nc.vector.tensor_add(vv[:, jt, :], pv[:], bv_bc[:])
                            # q^T, k^T: [e(128 part), i] = sum_d W[d,e] x^T[d,i]
                            for ec in range(DC):
                                esl = slice(ec * 128, (ec + 1) * 128)
                                pq = psbank.tile([128, 512], f32, tag="bank")
                                for dc in range(DC):
                                    nc.tensor.matmul(
                                        pq[:],
                                        wq_r[:, dc, esl],
                                        xT[:, dc, isl],
                                        start=(dc == 0),
                                        stop=(dc == DC - 1),
                                    )
                                nc.vector.tensor_scalar_add(
                                    qT[:, ec, isl], pq[:], bqT[:, ec : ec + 1]
                                )
                                pk = psbank.tile([128, 512], f32, tag="bank")
                                for dc in range(DC):
                                    nc.tensor.matmul(
                                        pk[:],
                                        wk_r[:, dc, esl],
                                        xT[:, dc, isl],
                                        start=(dc == 0),
                                        stop=(dc == DC - 1),
                                    )
                                nc.vector.tensor_scalar_add(
                                    kT[:, ec, isl], pk[:], bkT[:, ec : ec + 1]
                                )

                    # --- phase B: attention, one block of 512 queries at a time
                    if True:
                        for ib in range(NIB):
                            isl = slice(ib * 512, (ib + 1) * 512)
                            pT = pt_pool.tile([128, JT, 512], f32r)
                            for jt in range(JT):
                                jsl = slice(jt * 128, (jt + 1) * 128)
                                ps = psbank.tile([128, 512], f32, tag="bank")
                                for ec in range(DC):
                                    nc.tensor.matmul(
                                        ps[:],
                                        kT[:, ec, jsl],
                                        qT[:, ec, isl],
                                        start=(ec == 0),
                                        stop=(ec == DC - 1),
                                    )
                                nc.scalar.activation(
                                    pT[:, jt, :],
                                    ps[:],
                                    mybir.ActivationFunctionType.Exp,
                                )
                            # softmax denominators: s[1, i] = sum_j P^T[j, i].
                            # Pre-reduce 16 -> 4 tiles on DVE (idle during
                            # attention) to cut the PE ones-matmul count 4x.
                            red = red_pool.tile([128, 2, 512], f32r)
                            for g in range(2):
                                nc.vector.tensor_add(
                                    red[:, g, :],
                                    pT[:, 8 * g, :],
                                    pT[:, 8 * g + 1, :],
                                )
                                for j in range(8 * g + 2, 8 * g + 8):
                                    nc.vector.tensor_add(
                                        red[:, g, :], red[:, g, :], pT[:, j, :]
                                    )
                            sums_p = pssums.tile([1, 512], f32)
                            for g in range(2):
                                nc.tensor.matmul(
                                    sums_p[:],
                                    ones[:],
                                    red[:, g, :],
                                    start=(g == 0),
                                    stop=(g == 1),
                                )
                            s_sb = spool.tile([1, 512], f32)
                            nc.vector.tensor_copy(s_sb[:], sums_p[:])
                            st_p = pstrans.tile([128, 4], f32)
                            for c in range(4):
                                nc.tensor.transpose(
                                    st_p[:, c : c + 1],
                                    s_sb[0:1, c * 128 : (c + 1) * 128],
                                    ident[0:1, 0:1],
                                )
                            r_sb = rpool.tile([128, 4], f32, tag="r")
                            nc.vector.reciprocal(r_sb[:], st_p[:])

                            # out[i_sub] = (P^T)^T @ v, scaled by 1/s
                            for isub in range(4):
                                po = pspv.tile([128, 512], f32)
                                for jt in range(JT):
                                    nc.tensor.matmul(
                                        po[:],
                                        pT[:, jt, isub * 128 : (isub + 1) * 128],
                                        vv[:, jt, :],
                                        start=(jt == 0),
                                        stop=(jt == JT - 1),
                                    )
                                ob = ostage.tile([128, 512], f32, tag="ob")
                                nc.scalar.mul(ob[:], po[:], r_sb[:, isub : isub + 1])
                                t0 = ib * 512 + isub * 128
                                nc.sync.dma_start(
                                    out=out[b, t0 : t0 + 128, :], in_=ob[:]
                                )
    nc.finalize()
    return nc


_built = None


def kernel(x, Wq, bq, Wk, bk, Wv, bv):
    global _built
    x = np.ascontiguousarray(np.asarray(x, dtype=np.float32))
    ws = {
        "Wq": np.ascontiguousarray(np.asarray(Wq, dtype=np.float32)),
        "bq": np.ascontiguousarray(np.asarray(bq, dtype=np.float32)),
        "Wk": np.ascontiguousarray(np.asarray(Wk, dtype=np.float32)),
        "bk": np.ascontiguousarray(np.asarray(bk, dtype=np.float32)),
        "Wv": np.ascontiguousarray(np.asarray(Wv, dtype=np.float32)),
        "bv": np.ascontiguousarray(np.asarray(bv, dtype=np.float32)),
    }
    if _built is None:
        _built = build()
    in_maps = [
        {"x": np.ascontiguousarray(x[c * PB : (c + 1) * PB]), **ws}
        for c in range(NCORES)
    ]
    res = run_bass_kernel_spmd(_built, in_maps, core_ids=list(range(NCORES)))
    kernel.last_exec_time_ns = res.exec_time_ns
    return np.concatenate([r["out"] for r in res.results], axis=0)


kernel.last_exec_time_ns = None


# revision 30
# speedup vs baseline: 1.0987x; 1.0217x over previous
"""Single-head attention layer (Q/K/V proj + softmax(QK^T)V) on 8 trn2 NeuronCores.

Strategy: pure data-parallel over batch B=16 -> 2 batches per core, zero
communication. All matmuls run in float32r (fp32 storage, rounded fp32 PE mode,
1 cycle/row at free-dim>=512 => bf16-rate with ~tf32 precision).

Per core, per batch (x_b: [2048, 512]):
  1. x^T via PE transposes (d on partitions), rounded to f32r.
  2. q^T, k^T = (Wq/Wk)^T-contract projections in channel-major layout
     [e, token]; bias added per-partition during PSUM->SBUF copy.
     v = x @ Wv + bv in token-major layout [token, e].
  3. Scores computed transposed: S^T[j, i] = sum_e k^T[e,j] q^T[e,i],
     per i-block of 512 queries; exp (no max subtraction: |S| <~ 50, safe
     in fp32) written straight to SBUF as f32r => P^T ready for PV matmul.
  4. Softmax denominators: DVE pre-reduces the 16 P^T tiles to 2, then a
     ones-vector matmul sums over j partitions; tiny PE transposes land the
     sums on i-partitions, DVE reciprocal.
  5. out[i_tile] = P^T.T @ v accumulated over 16 j-tiles; normalization
     folded into the PSUM->SBUF copy (per-partition scale), DMA to DRAM.

Schedule notes (measured on HW): x DMAs split per 128-channel chunk across
queues; per 512-token window, v-projections run before q/k so the PE never
waits on the window's last x^T copy (done on ScalarE); Wv/bv load first since
v-projections consume them first; batch 1's transposes overlap batch 0's
attention because the xT pool region is freed early (pool open order).
"""

import os

import numpy as np

try:  # NTFF profiling hook is optional; without it, disable tracing so a
    # stray BASS_TRACE=1 in the environment cannot crash the run.
    from antenv.axon_hooks import get_axon_ntff_profile_hook  # noqa: F401
except ImportError:
    os.environ.setdefault("BASS_NEVER_TRACE", "1")

import concourse.bass as bass
import concourse.tile as tile
from concourse import bacc, mybir
from concourse.bass_utils import run_bass_kernel_spmd
from concourse.masks import make_identity

f32 = mybir.dt.float32
f32r = mybir.dt.float32r
bf16 = mybir.dt.bfloat16

B, N, D = 16, 2048, 512
NCORES = 8
PB = B // NCORES  # batches per core
NT = N // 128  # 16 token tiles
DC = D // 128  # 4 channel chunks of 128
NIB = N // 512  # 4 query blocks of 512
JT = NT  # 16 key tiles


def build():
    nc = bacc.Bacc("TRN2", target_bir_lowering=False, debug=False)

    x = nc.dram_tensor("x", [PB, N, D], f32, kind="ExternalInput")
    Wq = nc.dram_tensor("Wq", [D, D], f32, kind="ExternalInput")
    bq = nc.dram_tensor("bq", [D], f32, kind="ExternalInput")
    Wk = nc.dram_tensor("Wk", [D, D], f32, kind="ExternalInput")
    bk = nc.dram_tensor("bk", [D], f32, kind="ExternalInput")
    Wv = nc.dram_tensor("Wv", [D, D], f32, kind="ExternalInput")
    bv = nc.dram_tensor("bv", [D], f32, kind="ExternalInput")
    out = nc.dram_tensor("out", [PB, N, D], f32, kind="ExternalOutput")

    with tile.TileContext(nc) as tc:
        with (
            tc.tile_pool(name="singles", bufs=1) as singles,
            tc.tile_pool(name="psbank", bufs=4, space="PSUM") as psbank,
            tc.tile_pool(name="pstrans", bufs=1, space="PSUM") as pstrans,
            tc.tile_pool(name="pssums", bufs=1, space="PSUM") as pssums,
            tc.tile_pool(name="pspv", bufs=2, space="PSUM") as pspv,
            tc.tile_pool(name="spool", bufs=1) as spool,
            tc.tile_pool(name="xstage", bufs=5) as xstage,
            tc.tile_pool(name="rpool", bufs=1) as rpool,
        ):
            ident = singles.tile([128, 128], f32)
            make_identity(nc, ident[:])
            ones_f32 = singles.tile([128, 1], f32)
            nc.vector.memset(ones_f32[:], 1.0)
            ones = singles.tile([128, 1], f32r)
            nc.vector.tensor_copy(ones[:], ones_f32[:])

            # --- weights/biases load; emitted AFTER batch-0 x loads so the
            #     PE can start transposing x while weights stream in.
            wb = {}

            def load_weights():
                # stage weight chunks through the shared xstage slots
                for W in (Wv, Wq, Wk):
                    wr = singles.tile([128, DC, D], f32r, tag=f"w_{W.name}")
                    for dc in range(DC):
                        stage = xstage.tile([128, D], f32, tag="xs")
                        nc.sync.dma_start(
                            out=stage[:],
                            in_=W[dc * 128 : (dc + 1) * 128, :],
                        )
                        nc.vector.tensor_copy(wr[:, dc, :], stage[:])
                    wb[W.name] = wr
                # biases: bv (needed first) broadcast to all partitions;
                # bq/bk as [128, dc] (channel on partitions)
                bv_bc = singles.tile([128, D], f32)
                bv_ap = bv[:]
                bv_bcast = bass.AP(
                    tensor=bv_ap.tensor, offset=bv_ap.offset, ap=[[0, 128], *bv_ap.ap]
                )
                nc.sync.dma_start(out=bv_bc[:], in_=bv_bcast)
                bqT = singles.tile([128, DC], f32)
                nc.sync.dma_start(
                    out=bqT[:], in_=bq[:].rearrange("(dc p) -> p dc", p=128)
                )
                bkT = singles.tile([128, DC], f32)
                nc.sync.dma_start(
                    out=bkT[:], in_=bk[:].rearrange("(dc p) -> p dc", p=128)
                )
                wb["bqT"], wb["bkT"], wb["bv_bc"] = bqT, bkT, bv_bc

            for b in range(PB):
                with (
                    tc.tile_pool(name=f"qkv{b}", bufs=1) as qkv_pool,
                    tc.tile_pool(name=f"pT{b}", bufs=1) as pt_pool,
                    tc.tile_pool(name=f"red{b}", bufs=1) as red_pool,
                    tc.tile_pool(name=f"ostage{b}", bufs=2) as ostage,
                    tc.tile_pool(name=f"xT{b}", bufs=1) as xt_pool,
                ):
                    qT = qkv_pool.tile([128, DC, N], f32r, tag="qT")
                    kT = qkv_pool.tile([128, DC, N], f32r, tag="kT")
                    vv = qkv_pool.tile([128, NT, D], f32r, tag="v")

                    # --- phase A: x load, transpose, projections
                    if True:
                        xT = xt_pool.tile([128, DC, N], f32r)
                        # interleave per window of 4 token tiles (= one
                        # 512-wide projection block): DMA + transpose the
                        # window, then run its projections while the next
                        # window streams in. DMAs split per dc chunk so 4
                        # queues work each tile (lower latency per tile).
                        def stage_window(w):
                            for it in range(w * 4, w * 4 + 4):
                                xs = xstage.tile([128, D], f32, tag="xs")
                                tsl = slice(it * 128, (it + 1) * 128)
                                for dc in range(DC):
                                    csl = slice(dc * 128, (dc + 1) * 128)
                                    nc.sync.dma_start(
                                        out=xs[:, csl], in_=x[b, tsl, csl]
                                    )
                                ps = psbank.tile([128, DC, 128], f32, tag="bank")
                                for dc in range(DC):
                                    nc.tensor.transpose(
                                        ps[:, dc, :],
                                        xs[:, dc * 128 : (dc + 1) * 128],
                                        ident[:],
                                    )
                                nc.scalar.copy(xT[:, :, tsl], ps[:])

                        for ib in range(NIB):
                            stage_window(ib)
                            if b == 0 and ib == 0:
                                load_weights()
                            wq_r, wk_r, wv_r = wb["Wq"], wb["Wk"], wb["Wv"]
                            bqT, bkT, bv_bc = wb["bqT"], wb["bkT"], wb["bv_bc"]

                            isl = slice(ib * 512, (ib + 1) * 512)
                            # v first: v(jt) needs only tile jt, so it can run
                            # while the window's later xT copies land; q/k (which
                            # need the full window) go last, stall-free.
                            for jt in range(ib * 4, ib * 4 + 4):
                                jsl = slice(jt * 128, (jt + 1) * 128)
                                pv = psbank.tile([128, 512], f32, tag="bank")
                                for dc in range(DC):
                                    nc.tensor.matmul(
                                        pv[:],
                                        xT[:, dc, jsl],
                                        wv_r[:, dc, :],
                                        start=(dc == 0),
                                        stop=(dc == DC - 1),
                                    )
                                nc.vector.tensor_add(vv[:, jt, :], pv[:], bv_bc[:])
                            # q^T, k^T: [e(128 part), i] = sum_d W[d,e] x^T[d,i]
                            for ec in range(DC):
                                esl = slice(ec * 128, (ec + 1) * 128)
                                pq = psbank.tile([128, 512], f32, tag="bank")
                                for dc in range(DC):
                                    nc.tensor.matmul(
                                        pq[:],
                                        wq_r[:, dc, esl],
                                        xT[:, dc, isl],
                                        start=(dc == 0),
                                        stop=(dc == DC - 1),
                                    )
                                nc.vector.tensor_scalar_add(
                                    qT[:, ec, isl], pq[:], bqT[:, ec : ec + 1]
                                )
                                pk = psbank.tile([128, 512], f32, tag="bank")
                                for dc in range(DC):
                                    nc.tensor.matmul(
                                        pk[:],
                                        wk_r[:, dc, esl],
                                        xT[:, dc, isl],
                                        start=(dc == 0),
                                        stop=(dc == DC - 1),
                                    )
                                nc.vector.tensor_scalar_add(
                                    kT[:, ec, isl], pk[:], bkT[:, ec : ec + 1]
                                )

                    # --- phase B: attention, one block of 512 queries at a time
                    if True:
                        for ib in range(NIB):
                            isl = slice(ib * 512, (ib + 1) * 512)
                            pT = pt_pool.tile([128, JT, 512], f32r)
                            for jt in range(JT):
                                jsl = slice(jt * 128, (jt + 1) * 128)
                                ps = psbank.tile([128, 512], f32, tag="bank")
                                for ec in range(DC):
                                    nc.tensor.matmul(
                                        ps[:],
                                        kT[:, ec, jsl],
                                        qT[:, ec, isl],
                                        start=(ec == 0),
                                        stop=(ec == DC - 1),
                                    )
                                nc.scalar.activation(
                                    pT[:, jt, :],
                                    ps[:],
                                    mybir.ActivationFunctionType.Exp,
                                )
                            # softmax denominators: s[1, i] = sum_j P^T[j, i].
                            # Pre-reduce 16 -> 4 tiles on DVE (idle during
                            # attention) to cut the PE ones-matmul count 4x.
                            red = red_pool.tile([128, 2, 512], f32r)
                            for g in range(2):
                                nc.vector.tensor_add(
                                    red[:, g, :],
                                    pT[:, 8 * g, :],
                                    pT[:, 8 * g + 1, :],
                                )
                                for j in range(8 * g + 2, 8 * g + 8):
                                    nc.vector.tensor_add(
                                        red[:, g, :], red[:, g, :], pT[:, j, :]
                                    )
                            sums_p = pssums.tile([1, 512], f32)
                            for g in range(2):
                                nc.tensor.matmul(
                                    sums_p[:],
                                    ones[:],
                                    red[:, g, :],
                                    start=(g == 0),
                                    stop=(g == 1),
                                )
                            s_sb = spool.tile([1, 512], f32)
                            nc.vector.tensor_copy(s_sb[:], sums_p[:])
                            st_p = pstrans.tile([128, 4], f32)
                            for c in range(4):
                                nc.tensor.transpose(
                                    st_p[:, c : c + 1],
                                    s_sb[0:1, c * 128 : (c + 1) * 128],
                                    ident[0:1, 0:1],
                                )
                            r_sb = rpool.tile([128, 4], f32, tag="r")
                            nc.vector.reciprocal(r_sb[:], st_p[:])

                            # out[i_sub] = (P^T)^T @ v, scaled by 1/s
                            for isub in range(4):
                                po = pspv.tile([128, 512], f32)
                                for jt in range(JT):
                                    nc.tensor.matmul(
                                        po[:],
                                        pT[:, jt, isub * 128 : (isub + 1) * 128],
                                        vv[:, jt, :],
                                        start=(jt == 0),
                                        stop=(jt == JT - 1),
                                    )
                                ob = ostage.tile([128, 512], f32, tag="ob")
                                nc.scalar.mul(ob[:], po[:], r_sb[:, isub : isub + 1])
                                t0 = ib * 512 + isub * 128
                                nc.sync.dma_start(
                                    out=out[b, t0 : t0 + 128, :], in_=ob[:]
                                )
    nc.finalize()
    return nc


_built = None


def kernel(x, Wq, bq, Wk, bk, Wv, bv):
    global _built
    x = np.ascontiguousarray(np.asarray(x, dtype=np.float32))
    ws = {
        "Wq": np.ascontiguousarray(np.asarray(Wq, dtype=np.float32)),
        "bq": np.ascontiguousarray(np.asarray(bq, dtype=np.float32)),
        "Wk": np.ascontiguousarray(np.asarray(Wk, dtype=np.float32)),
        "bk": np.ascontiguousarray(np.asarray(bk, dtype=np.float32)),
        "Wv": np.ascontiguousarray(np.asarray(Wv, dtype=np.float32)),
        "bv": np.ascontiguousarray(np.asarray(bv, dtype=np.float32)),
    }
    if _built is None:
        _built = build()
    in_maps = [
        {"x": np.ascontiguousarray(x[c * PB : (c + 1) * PB]), **ws}
        for c in range(NCORES)
    ]
    res = run_bass_kernel_spmd(_built, in_maps, core_ids=list(range(NCORES)))
    kernel.last_exec_time_ns = res.exec_time_ns
    return np.concatenate([r["out"] for r in res.results], axis=0)


kernel.last_exec_time_ns = None


# revision 32
# speedup vs baseline: 1.1065x; 1.0071x over previous
"""Single-head attention layer (Q/K/V proj + softmax(QK^T)V) on 8 trn2 NeuronCores.

Strategy: pure data-parallel over batch B=16 -> 2 batches per core, zero
communication. All matmuls run in float32r (fp32 storage, rounded fp32 PE mode,
1 cycle/row at free-dim>=512 => bf16-rate with ~tf32 precision).

Per core, per batch (x_b: [2048, 512]):
  1. x^T via PE transposes (d on partitions), rounded to f32r.
  2. q^T, k^T = (Wq/Wk)^T-contract projections in channel-major layout
     [e, token]; bias added per-partition during PSUM->SBUF copy.
     v = x @ Wv + bv in token-major layout [token, e].
  3. Scores computed transposed: S^T[j, i] = sum_e k^T[e,j] q^T[e,i],
     per i-block of 512 queries; exp (no max subtraction: |S| <~ 50, safe
     in fp32) written straight to SBUF as f32r => P^T ready for PV matmul.
  4. Softmax denominators: DVE pre-reduces the 16 P^T tiles to 2, then a
     ones-vector matmul sums over j partitions; tiny PE transposes land the
     sums on i-partitions, DVE reciprocal.
  5. out[i_tile] = P^T.T @ v accumulated over 16 j-tiles; normalization
     folded into the PSUM->SBUF copy (per-partition scale), DMA to DRAM.

Schedule notes (measured on HW): x DMAs split per 128-channel chunk across
queues; per 512-token window, v-projections run before q/k so the PE never
waits on the window's last x^T copy (done on ScalarE); Wv/bv load first since
v-projections consume them first; batch 1's transposes overlap batch 0's
attention because the xT pool region is freed early (pool open order).
"""

import os

import numpy as np

try:  # NTFF profiling hook is optional; without it, disable tracing so a
    # stray BASS_TRACE=1 in the environment cannot crash the run.
    from antenv.axon_hooks import get_axon_ntff_profile_hook  # noqa: F401
except ImportError:
    os.environ.setdefault("BASS_NEVER_TRACE", "1")

import concourse.bass as bass
import concourse.tile as tile
from concourse import bacc, mybir
from concourse.bass_utils import run_bass_kernel_spmd
from concourse.masks import make_identity

f32 = mybir.dt.float32
f32r = mybir.dt.float32r
bf16 = mybir.dt.bfloat16

B, N, D = 16, 2048, 512
NCORES = 8
PB = B // NCORES  # batches per core
NT = N // 128  # 16 token tiles
DC = D // 128  # 4 channel chunks of 128
NIB = N // 512  # 4 query blocks of 512
JT = NT  # 16 key tiles


def build():
    nc = bacc.Bacc("TRN2", target_bir_lowering=False, debug=False)

    x = nc.dram_tensor("x", [PB, N, D], f32, kind="ExternalInput")
    Wq = nc.dram_tensor("Wq", [D, D], f32, kind="ExternalInput")
    bq = nc.dram_tensor("bq", [D], f32, kind="ExternalInput")
    Wk = nc.dram_tensor("Wk", [D, D], f32, kind="ExternalInput")
    bk = nc.dram_tensor("bk", [D], f32, kind="ExternalInput")
    Wv = nc.dram_tensor("Wv", [D, D], f32, kind="ExternalInput")
    bv = nc.dram_tensor("bv", [D], f32, kind="ExternalInput")
    out = nc.dram_tensor("out", [PB, N, D], f32, kind="ExternalOutput")

    with tile.TileContext(nc) as tc:
        with (
            tc.tile_pool(name="singles", bufs=1) as singles,
            tc.tile_pool(name="psbank", bufs=4, space="PSUM") as psbank,
            tc.tile_pool(name="pstrans", bufs=1, space="PSUM") as pstrans,
            tc.tile_pool(name="pssums", bufs=1, space="PSUM") as pssums,
            tc.tile_pool(name="pspv", bufs=2, space="PSUM") as pspv,
            tc.tile_pool(name="spool", bufs=1) as spool,
            tc.tile_pool(name="xstage", bufs=5) as xstage,
            tc.tile_pool(name="rpool", bufs=1) as rpool,
        ):
            ident = singles.tile([128, 128], f32)
            make_identity(nc, ident[:])
            ones_f32 = singles.tile([128, 1], f32)
            nc.vector.memset(ones_f32[:], 1.0)
            ones = singles.tile([128, 1], f32r)
            nc.vector.tensor_copy(ones[:], ones_f32[:])

            # --- weights/biases load; emitted AFTER batch-0 x loads so the
            #     PE can start transposing x while weights stream in.
            wb = {}

            def load_weights():
                # stage weight chunks through the shared xstage slots
                for W in (Wv, Wq, Wk):
                    wr = singles.tile([128, DC, D], f32r, tag=f"w_{W.name}")
                    for dc in range(DC):
                        stage = xstage.tile([128, D], f32, tag="xs")
                        nc.gpsimd.dma_start(
                            out=stage[:],
                            in_=W[dc * 128 : (dc + 1) * 128, :],
                        )
                        nc.vector.tensor_copy(wr[:, dc, :], stage[:])
                    wb[W.name] = wr
                # biases: bv (needed first) broadcast to all partitions;
                # bq/bk as [128, dc] (channel on partitions)
                bv_bc = singles.tile([128, D], f32)
                bv_ap = bv[:]
                bv_bcast = bass.AP(
                    tensor=bv_ap.tensor, offset=bv_ap.offset, ap=[[0, 128], *bv_ap.ap]
                )
                nc.gpsimd.dma_start(out=bv_bc[:], in_=bv_bcast)
                bqT = singles.tile([128, DC], f32)
                nc.gpsimd.dma_start(
                    out=bqT[:], in_=bq[:].rearrange("(dc p) -> p dc", p=128)
                )
                bkT = singles.tile([128, DC], f32)
                nc.gpsimd.dma_start(
                    out=bkT[:], in_=bk[:].rearrange("(dc p) -> p dc", p=128)
                )
                wb["bqT"], wb["bkT"], wb["bv_bc"] = bqT, bkT, bv_bc

            for b in range(PB):
                with (
                    tc.tile_pool(name=f"qkv{b}", bufs=1) as qkv_pool,
                    tc.tile_pool(name=f"pT{b}", bufs=1) as pt_pool,
                    tc.tile_pool(name=f"red{b}", bufs=1) as red_pool,
                    tc.tile_pool(name=f"ostage{b}", bufs=2) as ostage,
                    tc.tile_pool(name=f"xT{b}", bufs=1) as xt_pool,
                ):
                    qT = qkv_pool.tile([128, DC, N], f32r, tag="qT")
                    kT = qkv_pool.tile([128, DC, N], f32r, tag="kT")
                    vv = qkv_pool.tile([128, NT, D], f32r, tag="v")

                    # --- phase A: x load, transpose, projections
                    if True:
                        xT = xt_pool.tile([128, DC, N], f32r)
                        # interleave per window of 4 token tiles (= one
                        # 512-wide projection block): DMA + transpose the
                        # window, then run its projections while the next
                        # window streams in. DMAs split per dc chunk so 4
                        # queues work each tile (lower latency per tile).
                        def stage_window(w):
                            for it in range(w * 4, w * 4 + 4):
                                xs = xstage.tile([128, D], f32, tag="xs")
                                tsl = slice(it * 128, (it + 1) * 128)
                                for dc in range(DC):
                                    csl = slice(dc * 128, (dc + 1) * 128)
                                    nc.sync.dma_start(
                                        out=xs[:, csl], in_=x[b, tsl, csl]
                                    )
                                ps = psbank.tile([128, DC, 128], f32, tag="bank")
                                for dc in range(DC):
                                    nc.tensor.transpose(
                                        ps[:, dc, :],
                                        xs[:, dc * 128 : (dc + 1) * 128],
                                        ident[:],
                                    )
                                nc.scalar.copy(xT[:, :, tsl], ps[:])

                        for ib in range(NIB):
                            stage_window(ib)
                            if b == 0 and ib == 0:
                                load_weights()
                            wq_r, wk_r, wv_r = wb["Wq"], wb["Wk"], wb["Wv"]
                            bqT, bkT, bv_bc = wb["bqT"], wb["bkT"], wb["bv_bc"]

                            isl = slice(ib * 512, (ib + 1) * 512)
                            # v first: v(jt) needs only tile jt, so it can run
                            # while the window's later xT copies land; q/k (which
                            # need the full window) go last, stall-free.
                            for jt in range(ib * 4, ib * 4 + 4):
                                jsl = slice(jt * 128, (jt + 1) * 128)
                                pv = psbank.tile([128, 512], f32, tag="bank")
                                for dc in range(DC):
                                    nc.tensor.matmul(
                                        pv[:],
                                        xT[:, dc, jsl],
                                        wv_r[:, dc, :],
                                        start=(dc == 0),
                                        stop=(dc == DC - 1),
                                    )
                                nc.vector.tensor_add(vv[:, jt, :], pv[:], bv_bc[:])
                            # q^T, k^T: [e(128 part), i] = sum_d W[d,e] x^T[d,i]
                            for ec in range(DC):
                                esl = slice(ec * 128, (ec + 1) * 128)
                                pq = psbank.tile([128, 512], f32, tag="bank")
                                for dc in range(DC):
                                    nc.tensor.matmul(
                                        pq[:],
                                        wq_r[:, dc, esl],
                                        xT[:, dc, isl],
                                        start=(dc == 0),
                                        stop=(dc == DC - 1),
                                    )
                                nc.vector.tensor_scalar_add(
                                    qT[:, ec, isl], pq[:], bqT[:, ec : ec + 1]
                                )
                                pk = psbank.tile([128, 512], f32, tag="bank")
                                for dc in range(DC):
                                    nc.tensor.matmul(
                                        pk[:],
                                        wk_r[:, dc, esl],
                                        xT[:, dc, isl],
                                        start=(dc == 0),
                                        stop=(dc == DC - 1),
                                    )
                                nc.vector.tensor_scalar_add(
                                    kT[:, ec, isl], pk[:], bkT[:, ec : ec + 1]
                                )

                    # --- phase B: attention, one block of 512 queries at a time
                    if True:
                        for ib in range(NIB):
                            isl = slice(ib * 512, (ib + 1) * 512)
                            pT = pt_pool.tile([128, JT, 512], f32r)
                            for jt in range(JT):
                                jsl = slice(jt * 128, (jt + 1) * 128)
                                ps = psbank.tile([128, 512], f32, tag="bank")
                                for ec in range(DC):
                                    nc.tensor.matmul(
                                        ps[:],
                                        kT[:, ec, jsl],
                                        qT[:, ec, isl],
                                        start=(ec == 0),
                                        stop=(ec == DC - 1),
                                    )
                                nc.scalar.activation(
                                    pT[:, jt, :],
                                    ps[:],
                                    mybir.ActivationFunctionType.Exp,
                                )
                            # softmax denominators: s[1, i] = sum_j P^T[j, i].
                            # Pre-reduce 16 -> 4 tiles on DVE (idle during
                            # attention) to cut the PE ones-matmul count 4x.
                            red = red_pool.tile([128, 2, 512], f32r)
                            for g in range(2):
                                nc.vector.tensor_add(
                                    red[:, g, :],
                                    pT[:, 8 * g, :],
                                    pT[:, 8 * g + 1, :],
                                )
                                for j in range(8 * g + 2, 8 * g + 8):
                                    nc.vector.tensor_add(
                                        red[:, g, :], red[:, g, :], pT[:, j, :]
                                    )
                            sums_p = pssums.tile([1, 512], f32)
                            for g in range(2):
                                nc.tensor.matmul(
                                    sums_p[:],
                                    ones[:],
                                    red[:, g, :],
                                    start=(g == 0),
                                    stop=(g == 1),
                                )
                            s_sb = spool.tile([1, 512], f32)
                            nc.vector.tensor_copy(s_sb[:], sums_p[:])
                            st_p = pstrans.tile([128, 4], f32)
                            for c in range(4):
                                nc.tensor.transpose(
                                    st_p[:, c : c + 1],
                                    s_sb[0:1, c * 128 : (c + 1) * 128],
                                    ident[0:1, 0:1],
                                )
                            r_sb = rpool.tile([128, 4], f32, tag="r")
                            nc.vector.reciprocal(r_sb[:], st_p[:])

                            # out[i_sub] = (P^T)^T @ v, scaled by 1/s
                            for isub in range(4):
                                po = pspv.tile([128, 512], f32)
                                for jt in range(JT):
                                    nc.tensor.matmul(
                                        po[:],
                                        pT[:, jt, isub * 128 : (isub + 1) * 128],
                                        vv[:, jt, :],
                                        start=(jt == 0),
                                        stop=(jt == JT - 1),
                                    )
                                ob = ostage.tile([128, 512], f32, tag="ob")
                                nc.scalar.mul(ob[:], po[:], r_sb[:, isub : isub + 1])
                                t0 = ib * 512 + isub * 128
                                nc.gpsimd.dma_start(
                                    out=out[b, t0 : t0 + 128, :], in_=ob[:]
                                )
    nc.finalize()
    return nc


_built = None


def kernel(x, Wq, bq, Wk, bk, Wv, bv):
    global _built
    x = np.ascontiguousarray(np.asarray(x, dtype=np.float32))
    ws = {
        "Wq": np.ascontiguousarray(np.asarray(Wq, dtype=np.float32)),
        "bq": np.ascontiguousarray(np.asarray(bq, dtype=np.float32)),
        "Wk": np.ascontiguousarray(np.asarray(Wk, dtype=np.float32)),
        "bk": np.ascontiguousarray(np.asarray(bk, dtype=np.float32)),
        "Wv": np.ascontiguousarray(np.asarray(Wv, dtype=np.float32)),
        "bv": np.ascontiguousarray(np.asarray(bv, dtype=np.float32)),
    }
    if _built is None:
        _built = build()
    in_maps = [
        {"x": np.ascontiguousarray(x[c * PB : (c + 1) * PB]), **ws}
        for c in range(NCORES)
    ]
    res = run_bass_kernel_spmd(_built, in_maps, core_ids=list(range(NCORES)))
    kernel.last_exec_time_ns = res.exec_time_ns
    return np.concatenate([r["out"] for r in res.results], axis=0)


kernel.last_exec_time_ns = None


# revision 34
# speedup vs baseline: 1.1119x; 1.0049x over previous
"""Single-head attention layer (Q/K/V proj + softmax(QK^T)V) on 8 trn2 NeuronCores.

Strategy: pure data-parallel over batch B=16 -> 2 batches per core, zero
communication. All matmuls run in float32r (fp32 storage, rounded fp32 PE mode,
1 cycle/row at free-dim>=512 => bf16-rate with ~tf32 precision).

Per core, per batch (x_b: [2048, 512]):
  1. x^T via PE transposes (d on partitions), rounded to f32r.
  2. q^T, k^T = (Wq/Wk)^T-contract projections in channel-major layout
     [e, token]; bias added per-partition during PSUM->SBUF copy.
     v = x @ Wv + bv in token-major layout [token, e].
  3. Scores computed transposed: S^T[j, i] = sum_e k^T[e,j] q^T[e,i],
     per i-block of 512 queries; exp (no max subtraction: |S| <~ 50, safe
     in fp32) written straight to SBUF as f32r => P^T ready for PV matmul.
  4. Softmax denominators: DVE pre-reduces the 16 P^T tiles to 2, then a
     ones-vector matmul sums over j partitions; tiny PE transposes land the
     sums on i-partitions, DVE reciprocal.
  5. out[i_tile] = P^T.T @ v accumulated over 16 j-tiles; normalization
     folded into the PSUM->SBUF copy (per-partition scale), DMA to DRAM.

Schedule notes (measured on HW): x DMAs split per 128-channel chunk across
queues; per 512-token window, v-projections run before q/k so the PE never
waits on the window's last x^T copy (done on ScalarE); Wv/bv load first since
v-projections consume them first; batch 1's transposes overlap batch 0's
attention because the xT pool region is freed early (pool open order).
"""

import os

import numpy as np

try:  # NTFF profiling hook is optional; without it, disable tracing so a
    # stray BASS_TRACE=1 in the environment cannot crash the run.
    from antenv.axon_hooks import get_axon_ntff_profile_hook  # noqa: F401
except ImportError:
    os.environ.setdefault("BASS_NEVER_TRACE", "1")

import concourse.bass as bass
import concourse.tile as tile
from concourse import bacc, mybir
from concourse.bass_utils import run_bass_kernel_spmd
from concourse.masks import make_identity

f32 = mybir.dt.float32
f32r = mybir.dt.float32r
bf16 = mybir.dt.bfloat16

B, N, D = 16, 2048, 512
NCORES = 8
PB = B // NCORES  # batches per core
NT = N // 128  # 16 token tiles
DC = D // 128  # 4 channel chunks of 128
NIB = N // 512  # 4 query blocks of 512
JT = NT  # 16 key tiles


def build():
    nc = bacc.Bacc("TRN2", target_bir_lowering=False, debug=False)

    x = nc.dram_tensor("x", [PB, N, D], f32, kind="ExternalInput")
    Wq = nc.dram_tensor("Wq", [D, D], f32, kind="ExternalInput")
    bq = nc.dram_tensor("bq", [D], f32, kind="ExternalInput")
    Wk = nc.dram_tensor("Wk", [D, D], f32, kind="ExternalInput")
    bk = nc.dram_tensor("bk", [D], f32, kind="ExternalInput")
    Wv = nc.dram_tensor("Wv", [D, D], f32, kind="ExternalInput")
    bv = nc.dram_tensor("bv", [D], f32, kind="ExternalInput")
    out = nc.dram_tensor("out", [PB, N, D], f32, kind="ExternalOutput")

    with tile.TileContext(nc) as tc:
        with (
            tc.tile_pool(name="singles", bufs=1) as singles,
            tc.tile_pool(name="psbank", bufs=4, space="PSUM") as psbank,
            tc.tile_pool(name="pstrans", bufs=1, space="PSUM") as pstrans,
            tc.tile_pool(name="pssums", bufs=1, space="PSUM") as pssums,
            tc.tile_pool(name="pspv", bufs=2, space="PSUM") as pspv,
            tc.tile_pool(name="spool", bufs=1) as spool,
            tc.tile_pool(name="xstage", bufs=5) as xstage,
            tc.tile_pool(name="rpool", bufs=1) as rpool,
        ):
            ident = singles.tile([128, 128], f32)
            make_identity(nc, ident[:])
            ones_f32 = singles.tile([128, 1], f32)
            nc.vector.memset(ones_f32[:], 1.0)
            ones = singles.tile([128, 1], f32r)
            nc.vector.tensor_copy(ones[:], ones_f32[:])

            # --- weights/biases load; emitted AFTER batch-0 x loads so the
            #     PE can start transposing x while weights stream in.
            wb = {}

            def load_weights():
                # stage weight chunks through the shared xstage slots
                for W in (Wv, Wq, Wk):
                    wr = singles.tile([128, DC, D], f32r, tag=f"w_{W.name}")
                    for dc in range(DC):
                        stage = xstage.tile([128, D], f32, tag="xs")
                        nc.gpsimd.dma_start(
                            out=stage[:],
                            in_=W[dc * 128 : (dc + 1) * 128, :],
                        )
                        nc.vector.tensor_copy(wr[:, dc, :], stage[:])
                    wb[W.name] = wr
                # biases: bv (needed first) broadcast to all partitions;
                # bq/bk as [128, dc] (channel on partitions)
                bv_bc = singles.tile([128, D], f32)
                bv_ap = bv[:]
                bv_bcast = bass.AP(
                    tensor=bv_ap.tensor, offset=bv_ap.offset, ap=[[0, 128], *bv_ap.ap]
                )
                nc.gpsimd.dma_start(out=bv_bc[:], in_=bv_bcast)
                bqT = singles.tile([128, DC], f32)
                nc.gpsimd.dma_start(
                    out=bqT[:], in_=bq[:].rearrange("(dc p) -> p dc", p=128)
                )
                bkT = singles.tile([128, DC], f32)
                nc.gpsimd.dma_start(
                    out=bkT[:], in_=bk[:].rearrange("(dc p) -> p dc", p=128)
                )
                wb["bqT"], wb["bkT"], wb["bv_bc"] = bqT, bkT, bv_bc

            for b in range(PB):
                with (
                    tc.tile_pool(name=f"qkv{b}", bufs=1) as qkv_pool,
                    tc.tile_pool(name=f"pT{b}", bufs=1) as pt_pool,
                    tc.tile_pool(name=f"red{b}", bufs=1) as red_pool,
                    tc.tile_pool(name=f"ostage{b}", bufs=2) as ostage,
                    tc.tile_pool(name=f"xT{b}", bufs=1) as xt_pool,
                ):
                    qT = qkv_pool.tile([128, DC, N], f32r, tag="qT")
                    kT = qkv_pool.tile([128, DC, N], f32r, tag="kT")
                    vv = qkv_pool.tile([128, NT, D], f32r, tag="v")

                    # --- phase A: x load, transpose, projections
                    if True:
                        xT = xt_pool.tile([128, DC, N], f32r)
                        # interleave per window of 4 token tiles (= one
                        # 512-wide projection block): DMA + transpose the
                        # window, then run its projections while the next
                        # window streams in. DMAs split per dc chunk so 4
                        # queues work each tile (lower latency per tile).
                        def stage_window(w):
                            for it in range(w * 4, w * 4 + 4):
                                xs = xstage.tile([128, D], f32, tag="xs")
                                tsl = slice(it * 128, (it + 1) * 128)
                                for dc in range(DC):
                                    csl = slice(dc * 128, (dc + 1) * 128)
                                    nc.sync.dma_start(
                                        out=xs[:, csl], in_=x[b, tsl, csl]
                                    )
                                ps = psbank.tile([128, DC, 128], f32, tag="bank")
                                for dc in range(DC):
                                    nc.tensor.transpose(
                                        ps[:, dc, :],
                                        xs[:, dc * 128 : (dc + 1) * 128],
                                        ident[:],
                                    )
                                nc.scalar.copy(xT[:, :, tsl], ps[:])

                        for ib in range(NIB):
                            stage_window(ib)
                            if b == 0 and ib == 0:
                                load_weights()
                            wq_r, wk_r, wv_r = wb["Wq"], wb["Wk"], wb["Wv"]
                            bqT, bkT, bv_bc = wb["bqT"], wb["bkT"], wb["bv_bc"]

                            isl = slice(ib * 512, (ib + 1) * 512)
                            # v first: v(jt) needs only tile jt, so it can run
                            # while the window's later xT copies land; q/k (which
                            # need the full window) go last, stall-free.
                            for jt in range(ib * 4, ib * 4 + 4):
                                jsl = slice(jt * 128, (jt + 1) * 128)
                                pv = psbank.tile([128, 512], f32, tag="bank")
                                for dc in range(DC):
                                    nc.tensor.matmul(
                                        pv[:],
                                        xT[:, dc, jsl],
                                        wv_r[:, dc, :],
                                        start=(dc == 0),
                                        stop=(dc == DC - 1),
                                    )
                                nc.vector.tensor_add(vv[:, jt, :], pv[:], bv_bc[:])
                            # q^T, k^T: [e(128 part), i] = sum_d W[d,e] x^T[d,i]
                            for ec in range(DC):
                                esl = slice(ec * 128, (ec + 1) * 128)
                                pq = psbank.tile([128, 512], f32, tag="bank")
                                for dc in range(DC):
                                    nc.tensor.matmul(
                                        pq[:],
                                        wq_r[:, dc, esl],
                                        xT[:, dc, isl],
                                        start=(dc == 0),
                                        stop=(dc == DC - 1),
                                    )
                                nc.vector.tensor_scalar_add(
                                    qT[:, ec, isl], pq[:], bqT[:, ec : ec + 1]
                                )
                                pk = psbank.tile([128, 512], f32, tag="bank")
                                for dc in range(DC):
                                    nc.tensor.matmul(
                                        pk[:],
                                        wk_r[:, dc, esl],
                                        xT[:, dc, isl],
                                        start=(dc == 0),
                                        stop=(dc == DC - 1),
                                    )
                                nc.vector.tensor_scalar_add(
                                    kT[:, ec, isl], pk[:], bkT[:, ec : ec + 1]
                                )

                    # --- phase B: attention, one block of 512 queries at a time
                    if True:
                        for ib in range(NIB):
                            isl = slice(ib * 512, (ib + 1) * 512)
                            pT = pt_pool.tile([128, JT, 512], f32r)
                            for jt in range(JT):
                                jsl = slice(jt * 128, (jt + 1) * 128)
                                ps = psbank.tile([128, 512], f32, tag="bank")
                                for ec in range(DC):
                                    nc.tensor.matmul(
                                        ps[:],
                                        kT[:, ec, jsl],
                                        qT[:, ec, isl],
                                        start=(ec == 0),
                                        stop=(ec == DC - 1),
                                    )
                                nc.scalar.activation(
                                    pT[:, jt, :],
                                    ps[:],
                                    mybir.ActivationFunctionType.Exp,
                                )
                            # softmax denominators: s[1, i] = sum_j P^T[j, i].
                            # Pre-reduce 16 -> 4 tiles on DVE (idle during
                            # attention) to cut the PE ones-matmul count 4x.
                            red = red_pool.tile([128, 2, 512], f32r)
                            for g in range(2):
                                nc.vector.tensor_add(
                                    red[:, g, :],
                                    pT[:, 8 * g, :],
                                    pT[:, 8 * g + 1, :],
                                )
                                for j in range(8 * g + 2, 8 * g + 8):
                                    nc.vector.tensor_add(
                                        red[:, g, :], red[:, g, :], pT[:, j, :]
                                    )
                            sums_p = pssums.tile([1, 512], f32)
                            for g in range(2):
                                nc.tensor.matmul(
                                    sums_p[:],
                                    ones[:],
                                    red[:, g, :],
                                    start=(g == 0),
                                    stop=(g == 1),
                                )
                            s_sb = spool.tile([1, 512], f32)
                            nc.vector.tensor_copy(s_sb[:], sums_p[:])
                            st_p = pstrans.tile([128, 4], f32)
                            for c in range(4):
                                nc.tensor.transpose(
                                    st_p[:, c : c + 1],
                                    s_sb[0:1, c * 128 : (c + 1) * 128],
                                    ident[0:1, 0:1],
                                )
                            r_sb = rpool.tile([128, 4], f32, tag="r")
                            nc.vector.reciprocal(r_sb[:], st_p[:])

                            # out[i_sub] = (P^T)^T @ v, scaled by 1/s
                            for isub in range(4):
                                po = pspv.tile([128, 512], f32)
                                for jt in range(JT):
                                    nc.tensor.matmul(
                                        po[:],
                                        pT[:, jt, isub * 128 : (isub + 1) * 128],
                                        vv[:, jt, :],
                                        start=(jt == 0),
                                        stop=(jt == JT - 1),
                                    )
                                ob = ostage.tile([128, 512], f32, tag="ob")
                                nc.scalar.mul(ob[:], po[:], r_sb[:, isub : isub + 1])
                                t0 = ib * 512 + isub * 128
                                nc.gpsimd.dma_start(
                                    out=out[b, t0 : t0 + 128, :], in_=ob[:]
                                )
    nc.finalize()
    return nc


_built = None


def kernel(x, Wq, bq, Wk, bk, Wv, bv):
    global _built
    x = np.ascontiguousarray(np.asarray(x, dtype=np.float32))
    ws = {
        "Wq": np.ascontiguousarray(np.asarray(Wq, dtype=np.float32)),
        "bq": np.ascontiguousarray(np.asarray(bq, dtype=np.float32)),
        "Wk": np.ascontiguousarray(np.asarray(Wk, dtype=np.float32)),
        "bk": np.ascontiguousarray(np.asarray(bk, dtype=np.float32)),
        "Wv": np.ascontiguousarray(np.asarray(Wv, dtype=np.float32)),
        "bv": np.ascontiguousarray(np.asarray(bv, dtype=np.float32)),
    }
    if _built is None:
        _built = build()
    in_maps = [
        {"x": np.ascontiguousarray(x[c * PB : (c + 1) * PB]), **ws}
        for c in range(NCORES)
    ]
    res = run_bass_kernel_spmd(_built, in_maps, core_ids=list(range(NCORES)))
    kernel.last_exec_time_ns = res.exec_time_ns
    return np.concatenate([r["out"] for r in res.results], axis=0)


kernel.last_exec_time_ns = None


# revision 35
# speedup vs baseline: 1.1143x; 1.0021x over previous
"""Single-head attention layer (Q/K/V proj + softmax(QK^T)V) on 8 trn2 NeuronCores.

Strategy: pure data-parallel over batch B=16 -> 2 batches per core, zero
communication. All matmuls run in float32r (fp32 storage, rounded fp32 PE mode,
1 cycle/row at free-dim>=512 => bf16-rate with ~tf32 precision).

Per core, per batch (x_b: [2048, 512]):
  1. x^T via PE transposes (d on partitions), rounded to f32r.
  2. q^T, k^T = (Wq/Wk)^T-contract projections in channel-major layout
     [e, token]; bias added per-partition during PSUM->SBUF copy.
     v = x @ Wv + bv in token-major layout [token, e].
  3. Scores computed transposed: S^T[j, i] = sum_e k^T[e,j] q^T[e,i],
     per i-block of 512 queries; exp (no max subtraction: |S| <~ 50, safe
     in fp32) written straight to SBUF as f32r => P^T ready for PV matmul.
  4. Softmax denominators: DVE pre-reduces the 16 P^T tiles to 2, then a
     ones-vector matmul sums over j partitions; tiny PE transposes land the
     sums on i-partitions, DVE reciprocal.
  5. out[i_tile] = P^T.T @ v accumulated over 16 j-tiles; normalization
     folded into the PSUM->SBUF copy (per-partition scale), DMA to DRAM.

Schedule notes (measured on HW): x DMAs split per 128-channel chunk across
queues; per 512-token window, v-projections run before q/k so the PE never
waits on the window's last x^T copy (done on ScalarE); Wv/bv load first since
v-projections consume them first; batch 1's transposes overlap batch 0's
attention because the xT pool region is freed early (pool open order).
"""

import os

import numpy as np

try:  # NTFF profiling hook is optional; without it, disable tracing so a
    # stray BASS_TRACE=1 in the environment cannot crash the run.
    from antenv.axon_hooks import get_axon_ntff_profile_hook  # noqa: F401
except ImportError:
    os.environ.setdefault("BASS_NEVER_TRACE", "1")

import concourse.bass as bass
import concourse.tile as tile
from concourse import bacc, mybir
from concourse.bass_utils import run_bass_kernel_spmd
from concourse.masks import make_identity

f32 = mybir.dt.float32
f32r = mybir.dt.float32r
bf16 = mybir.dt.bfloat16

B, N, D = 16, 2048, 512
NCORES = 8
PB = B // NCORES  # batches per core
NT = N // 128  # 16 token tiles
DC = D // 128  # 4 channel chunks of 128
NIB = N // 512  # 4 query blocks of 512
JT = NT  # 16 key tiles


def build():
    nc = bacc.Bacc("TRN2", target_bir_lowering=False, debug=False)

    x = nc.dram_tensor("x", [PB, N, D], f32, kind="ExternalInput")
    Wq = nc.dram_tensor("Wq", [D, D], f32, kind="ExternalInput")
    bq = nc.dram_tensor("bq", [D], f32, kind="ExternalInput")
    Wk = nc.dram_tensor("Wk", [D, D], f32, kind="ExternalInput")
    bk = nc.dram_tensor("bk", [D], f32, kind="ExternalInput")
    Wv = nc.dram_tensor("Wv", [D, D], f32, kind="ExternalInput")
    bv = nc.dram_tensor("bv", [D], f32, kind="ExternalInput")
    out = nc.dram_tensor("out", [PB, N, D], f32, kind="ExternalOutput")

    with tile.TileContext(nc) as tc:
        with (
            tc.tile_pool(name="singles", bufs=1) as singles,
            tc.tile_pool(name="psbank", bufs=4, space="PSUM") as psbank,
            tc.tile_pool(name="pstrans", bufs=1, space="PSUM") as pstrans,
            tc.tile_pool(name="pssums", bufs=1, space="PSUM") as pssums,
            tc.tile_pool(name="pspv", bufs=2, space="PSUM") as pspv,
            tc.tile_pool(name="spool", bufs=1) as spool,
            tc.tile_pool(name="xstage", bufs=5) as xstage,
            tc.tile_pool(name="rpool", bufs=1) as rpool,
        ):
            ident = singles.tile([128, 128], f32)
            make_identity(nc, ident[:])
            ones_f32 = singles.tile([128, 1], f32)
            nc.vector.memset(ones_f32[:], 1.0)
            ones = singles.tile([128, 1], f32r)
            nc.vector.tensor_copy(ones[:], ones_f32[:])

            # --- weights/biases load; emitted AFTER batch-0 x loads so the
            #     PE can start transposing x while weights stream in.
            wb = {}

            def load_weights():
                # stage weight chunks through the shared xstage slots
                for W in (Wv, Wq, Wk):
                    wr = singles.tile([128, DC, D], f32r, tag=f"w_{W.name}")
                    for dc in range(DC):
                        stage = xstage.tile([128, D], f32, tag="xs")
                        nc.gpsimd.dma_start(
                            out=stage[:],
                            in_=W[dc * 128 : (dc + 1) * 128, :],
                        )
                        nc.vector.tensor_copy(wr[:, dc, :], stage[:])
                    wb[W.name] = wr
                # biases: bv (needed first) broadcast to all partitions;
                # bq/bk as [128, dc] (channel on partitions)
                bv_bc = singles.tile([128, D], f32)
                bv_ap = bv[:]
                bv_bcast = bass.AP(
                    tensor=bv_ap.tensor, offset=bv_ap.offset, ap=[[0, 128], *bv_ap.ap]
                )
                nc.gpsimd.dma_start(out=bv_bc[:], in_=bv_bcast)
                bqT = singles.tile([128, DC], f32)
                nc.gpsimd.dma_start(
                    out=bqT[:], in_=bq[:].rearrange("(dc p) -> p dc", p=128)
                )
                bkT = singles.tile([128, DC], f32)
                nc.gpsimd.dma_start(
                    out=bkT[:], in_=bk[:].rearrange("(dc p) -> p dc", p=128)
                )
                wb["bqT"], wb["bkT"], wb["bv_bc"] = bqT, bkT, bv_bc

            for b in range(PB):
                with (
                    tc.tile_pool(name=f"qkv{b}", bufs=1) as qkv_pool,
                    tc.tile_pool(name=f"pT{b}", bufs=1) as pt_pool,
                    tc.tile_pool(name=f"red{b}", bufs=1) as red_pool,
                    tc.tile_pool(name=f"ostage{b}", bufs=2) as ostage,
                    tc.tile_pool(name=f"xT{b}", bufs=1) as xt_pool,
                ):
                    qT = qkv_pool.tile([128, DC, N], f32r, tag="qT")
                    kT = qkv_pool.tile([128, DC, N], f32r, tag="kT")
                    vv = qkv_pool.tile([128, NT, D], f32r, tag="v")

                    # --- phase A: x load, transpose, projections
                    if True:
                        xT = xt_pool.tile([128, DC, N], f32r)
                        # interleave per window of 4 token tiles (= one
                        # 512-wide projection block): DMA + transpose the
                        # window, then run its projections while the next
                        # window streams in. DMAs split per dc chunk so 4
                        # queues work each tile (lower latency per tile).
                        def stage_window(w):
                            for it in range(w * 4, w * 4 + 4):
                                xs = xstage.tile([128, D], f32, tag="xs")
                                tsl = slice(it * 128, (it + 1) * 128)
                                for dc in range(DC):
                                    csl = slice(dc * 128, (dc + 1) * 128)
                                    nc.sync.dma_start(
                                        out=xs[:, csl], in_=x[b, tsl, csl]
                                    )
                                ps = psbank.tile([128, DC, 128], f32, tag="bank")
                                for dc in range(DC):
                                    nc.tensor.transpose(
                                        ps[:, dc, :],
                                        xs[:, dc * 128 : (dc + 1) * 128],
                                        ident[:],
                                    )
                                nc.scalar.copy(xT[:, :, tsl], ps[:])

                        for ib in range(NIB):
                            stage_window(ib)
                            if b == 0 and ib == 0:
                                load_weights()
                            wq_r, wk_r, wv_r = wb["Wq"], wb["Wk"], wb["Wv"]
                            bqT, bkT, bv_bc = wb["bqT"], wb["bkT"], wb["bv_bc"]

                            isl = slice(ib * 512, (ib + 1) * 512)
                            # v first: v(jt) needs only tile jt, so it can run
                            # while the window's later xT copies land; q/k (which
                            # need the full window) go last, stall-free.
                            for jt in range(ib * 4, ib * 4 + 4):
                                jsl = slice(jt * 128, (jt + 1) * 128)
                                pv = psbank.tile([128, 512], f32, tag="bank")
                                for dc in range(DC):
                                    nc.tensor.matmul(
                                        pv[:],
                                        xT[:, dc, jsl],
                                        wv_r[:, dc, :],
                                        start=(dc == 0),
                                        stop=(dc == DC - 1),
                                    )
                                nc.vector.tensor_add(vv[:, jt, :], pv[:], bv_bc[:])
                            # q^T, k^T: [e(128 part), i] = sum_d W[d,e] x^T[d,i]
                            for ec in range(DC):
                                esl = slice(ec * 128, (ec + 1) * 128)
                                pq = psbank.tile([128, 512], f32, tag="bank")
                                for dc in range(DC):
                                    nc.tensor.matmul(
                                        pq[:],
                                        wq_r[:, dc, esl],
                                        xT[:, dc, isl],
                                        start=(dc == 0),
                                        stop=(dc == DC - 1),
                                    )
                                nc.vector.tensor_scalar_add(
                                    qT[:, ec, isl], pq[:], bqT[:, ec : ec + 1]
                                )
                                pk = psbank.tile([128, 512], f32, tag="bank")
                                for dc in range(DC):
                                    nc.tensor.matmul(
                                        pk[:],
                                        wk_r[:, dc, esl],
                                        xT[:, dc, isl],
                                        start=(dc == 0),
                                        stop=(dc == DC - 1),
                                    )
                                nc.vector.tensor_scalar_add(
                                    kT[:, ec, isl], pk[:], bkT[:, ec : ec + 1]
                                )

                    # --- phase B: attention, one block of 512 queries at a time
                    if True:
                        for ib in range(NIB):
                            isl = slice(ib * 512, (ib + 1) * 512)
                            pT = pt_pool.tile([128, JT, 512], f32r)
                            for jt in range(JT):
                                jsl = slice(jt * 128, (jt + 1) * 128)
                                ps = psbank.tile([128, 512], f32, tag="bank")
                                for ec in range(DC):
                                    nc.tensor.matmul(
                                        ps[:],
                                        kT[:, ec, jsl],
                                        qT[:, ec, isl],
                                        start=(ec == 0),
                                        stop=(ec == DC - 1),
                                    )
                                nc.scalar.activation(
                                    pT[:, jt, :],
                                    ps[:],
                                    mybir.ActivationFunctionType.Exp,
                                )
                            # softmax denominators: s[1, i] = sum_j P^T[j, i].
                            # Pre-reduce 16 -> 4 tiles on DVE (idle during
                            # attention) to cut the PE ones-matmul count 4x.
                            red = red_pool.tile([128, 2, 512], f32r)
                            for g in range(2):
                                nc.vector.tensor_add(
                                    red[:, g, :],
                                    pT[:, 8 * g, :],
                                    pT[:, 8 * g + 1, :],
                                )
                                for j in range(8 * g + 2, 8 * g + 8):
                                    nc.vector.tensor_add(
                                        red[:, g, :], red[:, g, :], pT[:, j, :]
                                    )
                            nc.vector.tensor_add(
                                red[:, 0, :], red[:, 0, :], red[:, 1, :]
                            )
                            sums_p = pssums.tile([1, 512], f32)
                            nc.tensor.matmul(
                                sums_p[:],
                                ones[:],
                                red[:, 0, :],
                                start=True,
                                stop=True,
                            )
                            s_sb = spool.tile([1, 512], f32)
                            nc.vector.tensor_copy(s_sb[:], sums_p[:])
                            st_p = pstrans.tile([128, 4], f32)
                            for c in range(4):
                                nc.tensor.transpose(
                                    st_p[:, c : c + 1],
                                    s_sb[0:1, c * 128 : (c + 1) * 128],
                                    ident[0:1, 0:1],
                                )
                            r_sb = rpool.tile([128, 4], f32, tag="r")
                            nc.vector.reciprocal(r_sb[:], st_p[:])

                            # out[i_sub] = (P^T)^T @ v, scaled by 1/s
                            for isub in range(4):
                                po = pspv.tile([128, 512], f32)
                                for jt in range(JT):
                                    nc.tensor.matmul(
                                        po[:],
                                        pT[:, jt, isub * 128 : (isub + 1) * 128],
                                        vv[:, jt, :],
                                        start=(jt == 0),
                                        stop=(jt == JT - 1),
                                    )
                                ob = ostage.tile([128, 512], f32, tag="ob")
                                nc.scalar.mul(ob[:], po[:], r_sb[:, isub : isub + 1])
                                t0 = ib * 512 + isub * 128
                                nc.gpsimd.dma_start(
                                    out=out[b, t0 : t0 + 128, :], in_=ob[:]
                                )
    nc.finalize()
    return nc


_built = None


def kernel(x, Wq, bq, Wk, bk, Wv, bv):
    global _built
    x = np.ascontiguousarray(np.asarray(x, dtype=np.float32))
    ws = {
        "Wq": np.ascontiguousarray(np.asarray(Wq, dtype=np.float32)),
        "bq": np.ascontiguousarray(np.asarray(bq, dtype=np.float32)),
        "Wk": np.ascontiguousarray(np.asarray(Wk, dtype=np.float32)),
        "bk": np.ascontiguousarray(np.asarray(bk, dtype=np.float32)),
        "Wv": np.ascontiguousarray(np.asarray(Wv, dtype=np.float32)),
        "bv": np.ascontiguousarray(np.asarray(bv, dtype=np.float32)),
    }
    if _built is None:
        _built = build()
    in_maps = [
        {"x": np.ascontiguousarray(x[c * PB : (c + 1) * PB]), **ws}
        for c in range(NCORES)
    ]
    res = run_bass_kernel_spmd(_built, in_maps, core_ids=list(range(NCORES)))
    kernel.last_exec_time_ns = res.exec_time_ns
    return np.concatenate([r["out"] for r in res.results], axis=0)


kernel.last_exec_time_ns = None


# revision 37
# speedup vs baseline: 1.1261x; 1.0106x over previous
"""Single-head attention layer (Q/K/V proj + softmax(QK^T)V) on 8 trn2 NeuronCores.

Strategy: pure data-parallel over batch B=16 -> 2 batches per core, zero
communication. All matmuls run in float32r (fp32 storage, rounded fp32 PE mode,
1 cycle/row at free-dim>=512 => bf16-rate with ~tf32 precision).

Per core, per batch (x_b: [2048, 512]):
  1. x^T via PE transposes (d on partitions), rounded to f32r.
  2. q^T, k^T = (Wq/Wk)^T-contract projections in channel-major layout
     [e, token]; bias added per-partition during PSUM->SBUF copy.
     v = x @ Wv + bv in token-major layout [token, e].
  3. Scores computed transposed: S^T[j, i] = sum_e k^T[e,j] q^T[e,i],
     per i-block of 512 queries; exp (no max subtraction: |S| <~ 50, safe
     in fp32) written straight to SBUF as f32r => P^T ready for PV matmul.
  4. Softmax denominators: DVE pre-reduces the 16 P^T tiles to 2, then a
     ones-vector matmul sums over j partitions; tiny PE transposes land the
     sums on i-partitions, DVE reciprocal.
  5. out[i_tile] = P^T.T @ v accumulated over 16 j-tiles; normalization
     folded into the PSUM->SBUF copy (per-partition scale), DMA to DRAM.

Schedule notes (measured on HW): x DMAs split per 128-channel chunk across
queues; per 512-token window, v-projections run before q/k so the PE never
waits on the window's last x^T copy (done on ScalarE); Wv/bv load first since
v-projections consume them first; batch 1's transposes overlap batch 0's
attention because the xT pool region is freed early (pool open order).
"""

import os

import numpy as np

try:  # NTFF profiling hook is optional; without it, disable tracing so a
    # stray BASS_TRACE=1 in the environment cannot crash the run.
    from antenv.axon_hooks import get_axon_ntff_profile_hook  # noqa: F401
except ImportError:
    os.environ.setdefault("BASS_NEVER_TRACE", "1")

import concourse.bass as bass
import concourse.tile as tile
from concourse import bacc, mybir
from concourse.bass_utils import run_bass_kernel_spmd
from concourse.masks import make_identity

f32 = mybir.dt.float32
f32r = mybir.dt.float32r
bf16 = mybir.dt.bfloat16

B, N, D = 16, 2048, 512
NCORES = 8
PB = B // NCORES  # batches per core
NT = N // 128  # 16 token tiles
DC = D // 128  # 4 channel chunks of 128
NIB = N // 512  # 4 query blocks of 512
JT = NT  # 16 key tiles


def build():
    nc = bacc.Bacc("TRN2", target_bir_lowering=False, debug=False)

    x = nc.dram_tensor("x", [PB, N, D], f32, kind="ExternalInput")
    Wq = nc.dram_tensor("Wq", [D, D], f32, kind="ExternalInput")
    bq = nc.dram_tensor("bq", [D], f32, kind="ExternalInput")
    Wk = nc.dram_tensor("Wk", [D, D], f32, kind="ExternalInput")
    bk = nc.dram_tensor("bk", [D], f32, kind="ExternalInput")
    Wv = nc.dram_tensor("Wv", [D, D], f32, kind="ExternalInput")
    bv = nc.dram_tensor("bv", [D], f32, kind="ExternalInput")
    out = nc.dram_tensor("out", [PB, N, D], f32, kind="ExternalOutput")

    with tile.TileContext(nc) as tc:
        with (
            tc.tile_pool(name="singles", bufs=1) as singles,
            tc.tile_pool(name="psbank", bufs=4, space="PSUM") as psbank,
            tc.tile_pool(name="pstrans", bufs=1, space="PSUM") as pstrans,
            tc.tile_pool(name="pssums", bufs=1, space="PSUM") as pssums,
            tc.tile_pool(name="pspv", bufs=2, space="PSUM") as pspv,
            tc.tile_pool(name="spool", bufs=1) as spool,
            tc.tile_pool(name="xstage", bufs=5) as xstage,
            tc.tile_pool(name="rpool", bufs=1) as rpool,
        ):
            ident = singles.tile([128, 128], f32)
            make_identity(nc, ident[:])
            ones_f32 = singles.tile([128, 1], f32)
            nc.vector.memset(ones_f32[:], 1.0)
            ones = singles.tile([128, 1], f32r)
            nc.vector.tensor_copy(ones[:], ones_f32[:])

            # --- weights/biases load; emitted AFTER batch-0 x loads so the
            #     PE can start transposing x while weights stream in.
            wb = {}

            def load_weights():
                # stage weight chunks through the shared xstage slots
                for W in (Wv, Wq, Wk):
                    wr = singles.tile([128, DC, D], f32r, tag=f"w_{W.name}")
                    for dc in range(DC):
                        stage = xstage.tile([128, D], f32, tag="xs")
                        nc.gpsimd.dma_start(
                            out=stage[:],
                            in_=W[dc * 128 : (dc + 1) * 128, :],
                        )
                        nc.vector.tensor_copy(wr[:, dc, :], stage[:])
                    wb[W.name] = wr
                # biases: bv (needed first) broadcast to all partitions;
                # bq/bk as [128, dc] (channel on partitions)
                bv_bc = singles.tile([128, D], f32)
                bv_ap = bv[:]
                bv_bcast = bass.AP(
                    tensor=bv_ap.tensor, offset=bv_ap.offset, ap=[[0, 128], *bv_ap.ap]
                )
                nc.gpsimd.dma_start(out=bv_bc[:], in_=bv_bcast)
                bqT = singles.tile([128, DC], f32)
                nc.gpsimd.dma_start(
                    out=bqT[:], in_=bq[:].rearrange("(dc p) -> p dc", p=128)
                )
                bkT = singles.tile([128, DC], f32)
                nc.gpsimd.dma_start(
                    out=bkT[:], in_=bk[:].rearrange("(dc p) -> p dc", p=128)
                )
                wb["bqT"], wb["bkT"], wb["bv_bc"] = bqT, bkT, bv_bc

            for b in range(PB):
                with (
                    tc.tile_pool(name=f"qkv{b}", bufs=1) as qkv_pool,
                    tc.tile_pool(name=f"pT{b}", bufs=1) as pt_pool,
                    tc.tile_pool(name=f"red{b}", bufs=1) as red_pool,
                    tc.tile_pool(name=f"ostage{b}", bufs=2) as ostage,
                    tc.tile_pool(name=f"xT{b}", bufs=1) as xt_pool,
                ):
                    qT = qkv_pool.tile([128, DC, N], f32r, tag="qT")
                    kT = qkv_pool.tile([128, DC, N], f32r, tag="kT")
                    vv = qkv_pool.tile([128, NT, D], f32r, tag="v")

                    # --- phase A: x load, transpose, projections
                    if True:
                        xT = xt_pool.tile([128, DC, N], f32r)
                        # interleave per window of 4 token tiles (= one
                        # 512-wide projection block): DMA + transpose the
                        # window, then run its projections while the next
                        # window streams in. DMAs split per dc chunk so 4
                        # queues work each tile (lower latency per tile).
                        def stage_window(w):
                            for it in range(w * 4, w * 4 + 4):
                                xs = xstage.tile([128, D], f32, tag="xs")
                                t0 = it * 128
                                # row-split: 2 contiguous 64-row chunks -> two
                                # DMA queues per tile at half the DIRECT2D
                                # descriptor lines of a column split
                                nc.sync.dma_start(
                                    out=xs[0:64, :], in_=x[b, t0 : t0 + 64, :]
                                )
                                nc.sync.dma_start(
                                    out=xs[64:128, :],
                                    in_=x[b, t0 + 64 : t0 + 128, :],
                                )
                                ps = psbank.tile([128, DC, 128], f32, tag="bank")
                                for dc in range(DC):
                                    nc.tensor.transpose(
                                        ps[:, dc, :],
                                        xs[:, dc * 128 : (dc + 1) * 128],
                                        ident[:],
                                    )
                                nc.scalar.copy(xT[:, :, t0 : t0 + 128], ps[:])

                        for ib in range(NIB):
                            stage_window(ib)
                            if b == 0 and ib == 0:
                                load_weights()
                            wq_r, wk_r, wv_r = wb["Wq"], wb["Wk"], wb["Wv"]
                            bqT, bkT, bv_bc = wb["bqT"], wb["bkT"], wb["bv_bc"]

                            isl = slice(ib * 512, (ib + 1) * 512)
                            # v first: v(jt) needs only tile jt, so it can run
                            # while the window's later xT copies land; q/k (which
                            # need the full window) go last, stall-free.
                            for jt in range(ib * 4, ib * 4 + 4):
                                jsl = slice(jt * 128, (jt + 1) * 128)
                                pv = psbank.tile([128, 512], f32, tag="bank")
                                for dc in range(DC):
                                    nc.tensor.matmul(
                                        pv[:],
                                        xT[:, dc, jsl],
                                        wv_r[:, dc, :],
                                        start=(dc == 0),
                                        stop=(dc == DC - 1),
                                    )
                                nc.vector.tensor_add(vv[:, jt, :], pv[:], bv_bc[:])
                            # q^T, k^T: [e(128 part), i] = sum_d W[d,e] x^T[d,i]
                            for ec in range(DC):
                                esl = slice(ec * 128, (ec + 1) * 128)
                                pq = psbank.tile([128, 512], f32, tag="bank")
                                for dc in range(DC):
                                    nc.tensor.matmul(
                                        pq[:],
                                        wq_r[:, dc, esl],
                                        xT[:, dc, isl],
                                        start=(dc == 0),
                                        stop=(dc == DC - 1),
                                    )
                                nc.vector.tensor_scalar_add(
                                    qT[:, ec, isl], pq[:], bqT[:, ec : ec + 1]
                                )
                                pk = psbank.tile([128, 512], f32, tag="bank")
                                for dc in range(DC):
                                    nc.tensor.matmul(
                                        pk[:],
                                        wk_r[:, dc, esl],
                                        xT[:, dc, isl],
                                        start=(dc == 0),
                                        stop=(dc == DC - 1),
                                    )
                                nc.vector.tensor_scalar_add(
                                    kT[:, ec, isl], pk[:], bkT[:, ec : ec + 1]
                                )

                    # --- phase B: attention, one block of 512 queries at a time
                    if True:
                        for ib in range(NIB):
                            isl = slice(ib * 512, (ib + 1) * 512)
                            pT = pt_pool.tile([128, JT, 512], f32r)
                            for jt in range(JT):
                                jsl = slice(jt * 128, (jt + 1) * 128)
                                ps = psbank.tile([128, 512], f32, tag="bank")
                                for ec in range(DC):
                                    nc.tensor.matmul(
                                        ps[:],
                                        kT[:, ec, jsl],
                                        qT[:, ec, isl],
                                        start=(ec == 0),
                                        stop=(ec == DC - 1),
                                    )
                                nc.scalar.activation(
                                    pT[:, jt, :],
                                    ps[:],
                                    mybir.ActivationFunctionType.Exp,
                                )
                            # softmax denominators: s[1, i] = sum_j P^T[j, i].
                            # Pre-reduce 16 -> 4 tiles on DVE (idle during
                            # attention) to cut the PE ones-matmul count 4x.
                            red = red_pool.tile([128, 2, 512], f32r)
                            for g in range(2):
                                nc.vector.tensor_add(
                                    red[:, g, :],
                                    pT[:, 8 * g, :],
                                    pT[:, 8 * g + 1, :],
                                )
                                for j in range(8 * g + 2, 8 * g + 8):
                                    nc.vector.tensor_add(
                                        red[:, g, :], red[:, g, :], pT[:, j, :]
                                    )
                            nc.vector.tensor_add(
                                red[:, 0, :], red[:, 0, :], red[:, 1, :]
                            )
                            sums_p = pssums.tile([1, 512], f32)
                            nc.tensor.matmul(
                                sums_p[:],
                                ones[:],
                                red[:, 0, :],
                                start=True,
                                stop=True,
                            )
                            s_sb = spool.tile([1, 512], f32)
                            nc.vector.tensor_copy(s_sb[:], sums_p[:])
                            st_p = pstrans.tile([128, 4], f32)
                            for c in range(4):
                                nc.tensor.transpose(
                                    st_p[:, c : c + 1],
                                    s_sb[0:1, c * 128 : (c + 1) * 128],
                                    ident[0:1, 0:1],
                                )
                            r_sb = rpool.tile([128, 4], f32, tag="r")
                            nc.vector.reciprocal(r_sb[:], st_p[:])

                            # out[i_sub] = (P^T)^T @ v, scaled by 1/s
                            for isub in range(4):
                                po = pspv.tile([128, 512], f32)
                                for jt in range(JT):
                                    nc.tensor.matmul(
                                        po[:],
                                        pT[:, jt, isub * 128 : (isub + 1) * 128],
                                        vv[:, jt, :],
                                        start=(jt == 0),
                                        stop=(jt == JT - 1),
                                    )
                                ob = ostage.tile([128, 512], f32, tag="ob")
                                nc.scalar.mul(ob[:], po[:], r_sb[:, isub : isub + 1])
                                t0 = ib * 512 + isub * 128
                                nc.gpsimd.dma_start(
                                    out=out[b, t0 : t0 + 128, :], in_=ob[:]
                                )
    nc.finalize()
    return nc


_built = None


def kernel(x, Wq, bq, Wk, bk, Wv, bv):
    global _built
    x = np.ascontiguousarray(np.asarray(x, dtype=np.float32))
    ws = {
        "Wq": np.ascontiguousarray(np.asarray(Wq, dtype=np.float32)),
        "bq": np.ascontiguousarray(np.asarray(bq, dtype=np.float32)),
        "Wk": np.ascontiguousarray(np.asarray(Wk, dtype=np.float32)),
        "bk": np.ascontiguousarray(np.asarray(bk, dtype=np.float32)),
        "Wv": np.ascontiguousarray(np.asarray(Wv, dtype=np.float32)),
        "bv": np.ascontiguousarray(np.asarray(bv, dtype=np.float32)),
    }
    if _built is None:
        _built = build()
    in_maps = [
        {"x": np.ascontiguousarray(x[c * PB : (c + 1) * PB]), **ws}
        for c in range(NCORES)
    ]
    res = run_bass_kernel_spmd(_built, in_maps, core_ids=list(range(NCORES)))
    kernel.last_exec_time_ns = res.exec_time_ns
    return np.concatenate([r["out"] for r in res.results], axis=0)


kernel.last_exec_time_ns = None
